# revision 17
# baseline (speedup 1.0000x reference)
"""LIF spike-train kernel for Trainium2 (Bass/Tile), data-parallel over 8 cores.

Reference semantics (T=4, tau=0.5, thresh=1.0), per element:
    mem = 0
    for t in range(4):
        mem = mem*0.5 + x[t]
        s[t] = (mem - 1 >= 0)
        mem = mem - s[t]

x: [T*B, C, H, W] = [256, 128, 32, 32] f32, viewed as [4, 64, 128, 1024].
Batch dim (64) is sharded 8-ways; each core streams [4, 8, 128, 1024].

Two tricks vs the naive formulation:

1. Output compression: spikes are 0/1, so two timesteps are packed into one
   uint8 (s_t + 2*s_{t+1}) — store traffic drops 8x (16.8 MB -> 2.1 MB per
   core); the host unpacks bits back to f32 exactly.

2. Fused custom DVE ops. The DVE costs ~1 cycle/elem per tensor input, so
   the stock 13-op chain (~102us) drowns the ~52us load floor.  Tracking
   the PRE-reset membrane u_t (u_{t+1} = (u_t - (u_t>=1))*0.5 + x_{t+1},
   u_0 = x_0) lets one 2-input op advance a whole step, and one more emit
   two packed spike bits:
       LIF_U(u, x')   = (u - (u>=1))*C0 + x'
       SPIKE2B(u, x') = (u>=1) + C1*((u - (u>=1))*C0 + x' >= 1)  [uint8 out]
   Four ops per chunk (~9.2us) instead of thirteen -> DVE ~37us, back under
   the DMA roofline.  Every ALU stage is the same f32 is_ge/sub/mult/add the
   reference rounds through (mult by 0.5 and the subtract of 0/1 are exact),
   so the result stays bit-exact.
"""

import os
import sys

sys.path.insert(0, "/opt/trn_rl_repo")

import numpy as np

T = 4
B = 64
C = 128
HW = 1024
NCORES = 8
BLOC = B // NCORES  # 8 batch elements per core

LAST_EXEC_NS = None
LAST_TRACE = None

_CACHE = {}


def _register_ops():
    """Register the fused LIF ops in dve_ops.OPS (idempotent)."""
    import concourse.dve_ops as dvo
    from concourse.dve_spec import C0, C1, One, Spec, Src0, Src1, lower
    from concourse.dve_spec import _has_src1 as has_src1
    from concourse.dve_uop import DveOpSpec

    def reg(name, spec):
        if name in dvo._SUB_OPCODE_FOR_NAME:
            return next(o for o in dvo.OPS if o.name == name)
        shas = {}
        for ver in ("v3", "v4"):
            try:
                shas[ver] = DveOpSpec(
                    name=name,
                    opcode=dvo._CUSTOM_DVE_ROW_BASE + len(dvo.OPS),
                    uops=lower(spec, ver=ver),
                    rd1_en=has_src1(spec),
                ).sha(ver)
            except Exception:
                pass
        op = dvo.DveOp(name, spec, subdim=False, uops_sha=shas)
        dvo.OPS.append(op)
        dvo._SUB_OPCODE_FOR_NAME[name] = dvo._CUSTOM_DVE_ROW_BASE + len(dvo.OPS) - 1
        return op

    lif_u = reg(
        "LIF_U_ANT",
        Spec(
            body=(Src0 - (Src0 >= One)) * C0 + Src1,
            reference=lambda in0, in1, s0, s1, imm2: (
                (in0 - (in0 >= 1.0)) * s0 + in1
            ).astype(np.float32),
        ),
    )

    s_a = Src0 >= One
    u_n = (Src0 - s_a) * C0 + Src1
    s_b = u_n >= One

    def _spike2b_ref(in0, in1, s0, s1, imm2):
        a = (in0 >= 1.0).astype(np.float32)
        u = ((in0 - a) * s0 + in1).astype(np.float32)
        return a + s1 * (u >= 1.0)

    spike2b = reg("SPIKE2B_ANT", Spec(body=s_a + s_b * C1, reference=_spike2b_ref))
    return lif_u, spike2b


def _build(bloc=BLOC):
    """Per-core Bass module.  The computation is elementwise within a
    timestep, so each t-block [bloc, C, HW] is viewed as a flat [128, F]
    (F = bloc*C*HW/128): F*4-byte contiguous DRAM runs per partition give
    near-peak HBM bandwidth.  x: [T, 128, F] f32 in; yA/yB: [128, F] uint8
    out (yA bit0/bit1 = s0/s1, yB bit0/bit1 = s2/s3)."""
    import concourse.bacc as bacc
    import concourse.mybir as mybir
    from concourse import tile

    lif_u, spike2b = _register_ops()

    f32 = mybir.dt.float32
    u8 = mybir.dt.uint8

    F = bloc * C * HW // 128  # flat free width per t-block (8192 for bloc=8)
    W = min(int(os.environ.get("LIF_W", "2048")), F)  # chunk width
    NCH = F // W
    assert F % W == 0

    nc = bacc.Bacc("TRN2", target_bir_lowering=False, debug=False, num_devices=NCORES)
    x = nc.dram_tensor("x", [T, 128, F], f32, kind="ExternalInput").ap()
    ya = nc.dram_tensor("ya", [128, F], u8, kind="ExternalOutput").ap()
    yb = nc.dram_tensor("yb", [128, F], u8, kind="ExternalOutput").ap()

    # Loads split across both hardware DGE rings (SP + ACT); stores share
    # the rings but are deferred (see below).
    ld = [nc.sync, nc.scalar]

    if os.environ.get("LIF_SLIMTAIL", "1") == "1":
        from concourse.vector_clock import ScopedClock

        class _TileCls(tile.TileContext):
            """Keep the drain (stores must land) and the semaphore clears,
            but use a sem-only first barrier and drop the trailing
            all-engine barrier — NEFF completion already waits for every
            engine's stream end."""

            def _drain_and_barrier(self, tick_clock, wait_clock):
                drain_inst = self.nc.sync.drain()
                wait_clock.add_sem_waits(
                    drain_inst.ins, ScopedClock({None: tick_clock.global_clock})
                )
                self.nc.all_engine_barrier(sem_only=True)
                assert self.sems is not None
                popped = self.nc._tile_sem_poison_stack.pop()
                assert popped is self._sem_poison
                self.nc.clear_and_free_semaphores(
                    list(self.sems.allocated().values())
                )
    else:
        _TileCls = tile.TileContext

    tailsplit = os.environ.get("LIF_TAILSPLIT", "1") == "1"
    xbufs = int(os.environ.get("LIF_XBUFS", "12"))
    ubufs = int(os.environ.get("LIF_UBUFS", "4"))
    with _TileCls(nc) as tc:
        with tc.tile_pool(name="p", bufs=2) as pool:
            # A store dispatch carries the sem wait for its producing DVE op
            # and would block every later load dispatch on that ring, so each
            # chunk's stores are deferred until after the NEXT chunk's loads
            # are on the rings (by then the wait is long satisfied).
            defer = os.environ.get("LIF_DEFER", "1") == "1"
            pending = []
            for i in range(NCH):
                sl = slice(i * W, (i + 1) * W)
                xs = []
                for t in range(T):
                    xt = pool.tile([128, W], f32, tag="x", bufs=xbufs)
                    ld[(i * T + t) % 2].dma_start(out=xt, in_=x[t][:, sl])
                    xs.append(xt)
                for eng, out_ap, tile_ap in pending:
                    eng.dma_start(out=out_ap, in_=tile_ap)
                pending = []

                u1 = pool.tile([128, W], f32, tag="u", bufs=ubufs)
                nc.vector._custom_dve(lif_u, out=u1, in0=xs[0], in1=xs[1], s0=0.5)
                a8 = pool.tile([128, W], u8, tag="a8", bufs=2)
                nc.vector._custom_dve(
                    spike2b, out=a8, in0=xs[0], in1=xs[1], s0=0.5, s1=2.0
                )
                u2 = pool.tile([128, W], f32, tag="u", bufs=ubufs)
                nc.vector._custom_dve(lif_u, out=u2, in0=u1, in1=xs[2], s0=0.5)
                b8 = pool.tile([128, W], u8, tag="b8", bufs=2)
                if tailsplit and i == NCH - 1:
                    # Final chunk: quarter the last op so its store starts
                    # after W/4 of compute instead of serializing behind
                    # the whole tile.
                    h = W // 4
                    for k in range(4):
                        ss = slice(k * h, (k + 1) * h)
                        nc.vector._custom_dve(
                            spike2b,
                            out=b8[:, ss],
                            in0=u2[:, ss],
                            in1=xs[3][:, ss],
                            s0=0.5,
                            s1=2.0,
                        )
                        ld[k % 2].dma_start(
                            out=yb[:, i * W + k * h : i * W + (k + 1) * h],
                            in_=b8[:, ss],
                        )
                    ld[0].dma_start(out=ya[:, sl], in_=a8)
                else:
                    nc.vector._custom_dve(
                        spike2b, out=b8, in0=u2, in1=xs[3], s0=0.5, s1=2.0
                    )
                    if defer:
                        pending.append((ld[1], yb[:, sl], b8))
                        pending.append((ld[0], ya[:, sl], a8))
                    else:
                        ld[1].dma_start(out=yb[:, sl], in_=b8)
                        ld[0].dma_start(out=ya[:, sl], in_=a8)

    nc.compile()
    return nc


def _build_raw(bloc=BLOC):
    """Raw bacc variant: hand-rolled semaphores (12 vs Tile's ~45), no Tile
    preamble/teardown.  Same dataflow as _build(): per chunk, loads x0..x3
    split over the SP/ACT rings, DVE runs u1 = LIF_U(x0,x1),
    a8 = SPIKE2B(x0,x1), u2 = LIF_U(u1,x2), b8 = SPIKE2B(u2,x3), and the
    packed uint8 tiles are stored with each chunk's stores dispatched after
    the next chunk's loads.  U1/U2 are single buffers (only the in-order
    DVE touches them); A8/B8 double-buffer against store completion."""
    import concourse.bacc as bacc
    import concourse.mybir as mybir

    lif_u, spike2b = _register_ops()

    f32 = mybir.dt.float32
    u8 = mybir.dt.uint8

    F = bloc * C * HW // 128
    W = min(int(os.environ.get("LIF_W", "2048")), F)
    NCH = F // W
    assert F % W == 0
    NX = 2 * T  # x ring slots, two chunks deep

    nc = bacc.Bacc("TRN2", target_bir_lowering=False, debug=False, num_devices=NCORES)
    x = nc.dram_tensor("x", [T, 128, F], f32, kind="ExternalInput").ap()
    ya = nc.dram_tensor("ya", [128, F], u8, kind="ExternalOutput").ap()
    yb = nc.dram_tensor("yb", [128, F], u8, kind="ExternalOutput").ap()

    X = [nc.alloc_sbuf_tensor(f"X{k}", [128, W], f32).ap() for k in range(NX)]
    U1 = nc.alloc_sbuf_tensor("U1", [128, W], f32).ap()
    U2 = nc.alloc_sbuf_tensor("U2", [128, W], f32).ap()
    A8 = [nc.alloc_sbuf_tensor(f"A8_{k}", [128, W], u8).ap() for k in range(2)]
    B8 = [nc.alloc_sbuf_tensor(f"B8_{k}", [128, W], u8).ap() for k in range(2)]

    from contextlib import ExitStack

    with ExitStack() as stack:
        block = stack.enter_context(nc.Block(no_gpsimd_drain=True))
        xs = [stack.enter_context(nc.semaphore(f"xs{k}")) for k in range(NX)]
        cf = stack.enter_context(nc.semaphore("cf"))  # DVE op completions
        sa = stack.enter_context(nc.semaphore("sa"))  # ya store packets
        sb = stack.enter_context(nc.semaphore("sb"))  # yb store packets

        # cf counts DVE op completions (4 per chunk: u1, a8, u2, b8).  The
        # x tiles of chunk j are dead after: a8(j) for t0/t1 (cf=4j+2),
        # u2(j) for t2 (cf=4j+3), b8(j) for t3 (cf=4j+4).
        def loads(eng, i, ts):
            for t in ts:
                g = 4 * i + t  # global load index
                slot = g % NX
                if g >= NX:
                    j = i - 2
                    eng.wait_ge(cf, 4 * j + (2 if t < 2 else t + 1))
                eng.dma_start(
                    out=X[slot], in_=x[t][:, i * W : (i + 1) * W]
                ).then_inc(xs[slot], 16)

        @block.sync
        def _(sp: object):
            for i in range(NCH):
                loads(sp, i, (0, 2))
                if i >= 1:
                    sp.wait_ge(cf, 4 * (i - 1) + 2)
                    sp.dma_start(
                        out=ya[:, (i - 1) * W : i * W], in_=A8[(i - 1) % 2]
                    ).then_inc(sa, 16)
            sp.wait_ge(cf, 4 * (NCH - 1) + 2)
            sp.dma_start(
                out=ya[:, (NCH - 1) * W : NCH * W], in_=A8[(NCH - 1) % 2]
            ).then_inc(sa, 16)
            sp.wait_ge(sa, 16 * NCH)

        @block.scalar
        def _(act: object):
            for i in range(NCH):
                loads(act, i, (1, 3))
                if i >= 1:
                    act.wait_ge(cf, 4 * (i - 1) + 4)
                    act.dma_start(
                        out=yb[:, (i - 1) * W : i * W], in_=B8[(i - 1) % 2]
                    ).then_inc(sb, 16)
            h = W // 2
            base = (NCH - 1) * W
            act.wait_ge(cf, 4 * NCH)
            act.dma_start(
                out=yb[:, base : base + h], in_=B8[(NCH - 1) % 2][:, :h]
            ).then_inc(sb, 16)
            act.wait_ge(cf, 4 * NCH + 1)
            act.dma_start(
                out=yb[:, base + h : NCH * W], in_=B8[(NCH - 1) % 2][:, h:]
            ).then_inc(sb, 16)
            act.wait_ge(sb, 16 * (NCH + 1))

        @block.vector
        def _(ve: object):
            for i in range(NCH):
                s0, s1, s2, s3 = (
                    (4 * i) % NX,
                    (4 * i + 1) % NX,
                    (4 * i + 2) % NX,
                    (4 * i + 3) % NX,
                )
                p = 16 * (i // 2 + 1)  # packet target for this slot pass
                ve.wait_ge(xs[s0], p)
                ve.wait_ge(xs[s1], p)
                ve._custom_dve(
                    lif_u, out=U1, in0=X[s0], in1=X[s1], s0=0.5
                ).then_inc(cf, 1)
                if i >= 2:
                    ve.wait_ge(sa, 16 * (i - 1))
                ve._custom_dve(
                    spike2b, out=A8[i % 2], in0=X[s0], in1=X[s1], s0=0.5, s1=2.0
                ).then_inc(cf, 1)
                ve.wait_ge(xs[s2], p)
                ve._custom_dve(
                    lif_u, out=U2, in0=U1, in1=X[s2], s0=0.5
                ).then_inc(cf, 1)
                ve.wait_ge(xs[s3], p)
                if i >= 2:
                    ve.wait_ge(sb, 16 * (i - 1))
                if i == NCH - 1:
                    # Final chunk: halve the last op so the store of the
                    # first half overlaps the second half's compute.
                    h = W // 2
                    ve._custom_dve(
                        spike2b,
                        out=B8[i % 2][:, :h],
                        in0=U2[:, :h],
                        in1=X[s3][:, :h],
                        s0=0.5,
                        s1=2.0,
                    ).then_inc(cf, 1)
                    ve._custom_dve(
                        spike2b,
                        out=B8[i % 2][:, h:],
                        in0=U2[:, h:],
                        in1=X[s3][:, h:],
                        s0=0.5,
                        s1=2.0,
                    ).then_inc(cf, 1)
                else:
                    ve._custom_dve(
                        spike2b, out=B8[i % 2], in0=U2, in1=X[s3], s0=0.5, s1=2.0
                    ).then_inc(cf, 1)

        @block.gpsimd
        def _(gp: object):
            # Reset every sem for the next NEFF execution (range ops, 2 insts)
            gp.wait_ge(sa, 16 * NCH)
            gp.wait_ge(sb, 16 * (NCH + 1))
            rng = range(xs[0].num, sb.num + 1)
            assert sb.num - xs[0].num == NX + 2, (xs[0].num, sb.num)
            gp.dma_reset(rng)
            gp.sem_clear(rng)

    nc.compile()
    return nc


def _get_nc():
    if "nc" not in _CACHE:
        builder = _build if os.environ.get("LIF_RAW", "1") != "1" else _build_raw
        _CACHE["nc"] = builder()
    return _CACHE["nc"]


def kernel(x: np.ndarray) -> np.ndarray:
    global LAST_EXEC_NS, LAST_TRACE
    from concourse.bass_utils import run_bass_kernel_spmd

    x = np.ascontiguousarray(np.asarray(x), dtype=np.float32)
    assert x.shape == (T * B, C, 32, 32), x.shape
    xv = x.reshape(T, B, C, HW)

    F = BLOC * C * HW // 128
    in_maps = []
    for m in range(NCORES):
        shard = np.ascontiguousarray(xv[:, m * BLOC : (m + 1) * BLOC]).reshape(
            T, 128, F
        )
        in_maps.append({"x": shard})

    nc = _get_nc()
    trace = os.environ.get("LIF_TRACE") == "1"
    res = run_bass_kernel_spmd(nc, in_maps, core_ids=list(range(NCORES)), trace=trace)
    LAST_EXEC_NS = res.exec_time_ns
    if res.instructions_and_trace is not None:
        LAST_TRACE = res.instructions_and_trace[1]

    out = np.empty((T, B, C, HW), dtype=np.float32)
    for m in range(NCORES):
        bs = slice(m * BLOC, (m + 1) * BLOC)
        pa = res.results[m]["ya"].reshape(BLOC, C, HW)
        pb = res.results[m]["yb"].reshape(BLOC, C, HW)
        out[0, bs] = pa & 1
        out[1, bs] = (pa >> 1) & 1
        out[2, bs] = pb & 1
        out[3, bs] = (pb >> 1) & 1
    return out.reshape(T * B, C, 32, 32)
